# revision 19
# baseline (speedup 1.0000x reference)
"""Trainium2 Bass kernel for CompactPiecewiseLinearEmbeddings.

out[n, f*8+d] = sum_b h[n,f,b] * W[f,b,d] + b[f,d]
h = piecewise-linear encoding of x[n,f] over per-feature bins
    (first bin clamp_max(1), middle clamp(0,1), last bin clamp_min(0)).

Strategy (per core; data-parallel over N across 8 cores):
 - All-bf16 PE path (f32r moving data caps the PE clock at 1.2 GHz;
   pure-bf16 streams sustain 2.4 GHz).  x is split hi/lo into two bf16
   rows per feature (bf16*bf16 products are exact in the fp32 PSUM
   accumulate), and the per-bin bias -e*winv is split across two bf16
   ones-rows, so stage-1 matches f32r accuracy.
 - Host packs x into 8 xT tiles [128, NS]: two 34-row bands per tile
   (rows 0/64 +: 16 x_hi, 16 x_lo, ones, ones) serving groups (2i,2i+1).
 - Stage-1 bf16 matmul per (group, tile): s[f,j] = winv*x - e*winv.
   Contraction is always the full 128 partitions with zeros in the
   unused weight rows: mixing partial-band (tile_position) matmuls with
   full-128 ones drops the PE cadence from 216ns to ~322ns per matmul
   (measured), while uniform [128,128,512] shapes sustain the 2.4 GHz
   boost.
 - Clamp s -> h (bf16) via two static routes:
     A (~37%): DVE dual tensor_scalar (max, min) from PSUM.
     B: ACT Relu from PSUM (bin0 rows use the negated relu((e1-x)*winv)
        form with +W0 folded into the output bias), then DVE min-pass.
 - Stage-2 bf16 matmul contracts h against block-diagonal W into
   [128 fd, 1024] PSUM (6 accumulating matmuls per group).
 - Output bias+evac split ACT/DVE (3:1) -> bf16 SBUF -> DMA out in
   [fd, n] layout.  Host transposes/casts to the final [n, fd] f32.
"""
import numpy as np
import ml_dtypes

from concourse import bacc, mybir
from concourse.tile import TileContext
from concourse.bass_utils import run_bass_kernel_spmd

N, F, B, D = 16384, 256, 48, 8
NCORES = 8
NS = N // NCORES          # 2048 rows per core
CH = 512                  # matmul free dim (PSUM bank = 512 f32)
NG = 16                   # feature groups (16 features each)
TPG = 6                   # h-tiles per group (16*48/128)
NT = NG * TPG             # 96 h-tiles
NB = 34                   # band rows: 16 x_hi + 16 x_lo + 2 ones
BIG = 1e30

BF16 = ml_dtypes.bfloat16


def tile_route_a(gi):
    """Static clamp route per h-tile: A = DVE dual clamp, B = ACT relu
    + DVE min.  40% A, evenly spread so DVE-heavy tiles never cluster
    (clusters stall the PE on the ps-ring)."""
    return (gi % 10) in (0, 3, 5, 8)


def bfr(a):
    """Round f32 array to bf16 grid, keep f32."""
    return np.asarray(a, np.float32).astype(BF16).astype(np.float32)


_cache = {}


def build_nc():
    nc = bacc.Bacc("TRN2")
    f32, bf16 = mybir.dt.float32, mybir.dt.bfloat16

    xT_ext = nc.declare_dram_parameter("xT", [8 * 128, NS], bf16, isOutput=False)
    selpk_ext = nc.declare_dram_parameter("selpk", [128, NT * 128], bf16,
                                          isOutput=False)
    wpk_ext = nc.declare_dram_parameter("wpack", [128, NT * 128], bf16,
                                        isOutput=False)
    obias_ext = nc.declare_dram_parameter("obias", [128, NG], f32, isOutput=False)
    maxv_ext = nc.declare_dram_parameter("maxv", [128, NT], f32, isOutput=False)
    minv_ext = nc.declare_dram_parameter("minv", [128, NT], f32, isOutput=False)
    out_ext = nc.declare_dram_parameter("out", [F * D, NS], bf16, isOutput=True)

    Ident = mybir.ActivationFunctionType.Identity
    Relu = mybir.ActivationFunctionType.Relu
    amax, amin = mybir.AluOpType.max, mybir.AluOpType.min
    aadd = mybir.AluOpType.add

    with TileContext(nc) as tc:
        with (
            tc.tile_pool(name="const", bufs=1) as cpool,
            tc.tile_pool(name="hbuf", bufs=14) as hpool,
            tc.tile_pool(name="sbuf2", bufs=8) as spool,
            tc.tile_pool(name="osb", bufs=4) as opool,
            tc.tile_pool(name="bc", bufs=3, space="PSUM") as bcpool,
            tc.tile_pool(name="oc", bufs=1, space="PSUM") as ocpool,
        ):
            # ---- constants ----
            xT = [cpool.tile([128, NS], bf16, tag=f"xT{i}", name=f"xT{i}")
                  for i in range(8)]
            # chunked const tiles, DMA-ordered by first use (deps are
            # tile-granular: one big tile would stall the first matmul on
            # the whole transfer)
            NCHK = NT // 4                      # 24 h-tile blocks per chunk
            selpk4 = [cpool.tile([128, NCHK * 128], bf16, tag=f"selpk{i}",
                                 name=f"selpk{i}") for i in range(4)]
            wpk4 = [cpool.tile([128, NCHK * 128], bf16, tag=f"wpk{i}",
                               name=f"wpk{i}") for i in range(4)]
            obias = cpool.tile([128, NG], f32)
            maxv = cpool.tile([128, NT], f32)
            minv = cpool.tile([128, NT], f32)
            CW = NCHK * 128

            def dma_blk(dst, ext, chunk, lo, n):
                """DMA blocks [lo, lo+n) of a chunk tile."""
                nc.sync.dma_start(
                    out=dst[:, lo * 128:(lo + n) * 128],
                    in_=ext[:, (chunk * NCHK + lo) * 128:
                            (chunk * NCHK + lo + n) * 128])

            def dma_xt(i, half, nsl=1):
                """DMA one column half of an xT tile (sliced across nsl
                queues); half 1 feeds only the second cp iteration and
                can arrive late."""
                HN = NS // 2
                w = HN // nsl
                for s in range(nsl):
                    lo = half * HN + s * w
                    nc.sync.dma_start(
                        out=xT[i][:, lo:lo + w],
                        in_=xT_ext[i * 128:(i + 1) * 128, lo:lo + w])

            # critical-first order: the first group needs maxv/minv,
            # selpk blocks 0-5, xT0 cols 0:1024, wpk blocks 0-5; per-queue
            # bandwidth is ~40 GB/s so critical transfers are sliced across
            # several queues
            dma_blk(selpk4[0], selpk_ext, 0, 0, 3)
            dma_xt(0, 0, 4)
            dma_blk(selpk4[0], selpk_ext, 0, 3, 3)
            nc.sync.dma_start(out=maxv[:], in_=maxv_ext[:])
            nc.sync.dma_start(out=minv[:], in_=minv_ext[:])
            dma_blk(wpk4[0], wpk_ext, 0, 0, 3)
            dma_blk(wpk4[0], wpk_ext, 0, 3, 3)
            nc.sync.dma_start(out=obias[:], in_=obias_ext[:])
            dma_xt(1, 0, 2)
            dma_blk(selpk4[0], selpk_ext, 0, 6, 9)
            dma_blk(selpk4[0], selpk_ext, 0, 15, 9)
            dma_blk(wpk4[0], wpk_ext, 0, 6, 9)
            dma_blk(wpk4[0], wpk_ext, 0, 15, 9)
            for i in (1, 2, 3):
                dma_blk(selpk4[i], selpk_ext, i, 0, NCHK)
                dma_xt(2 * i, 0)
                dma_blk(wpk4[i], wpk_ext, i, 0, NCHK)
                dma_xt(2 * i + 1, 0)
            for i in range(8):
                dma_xt(i, 1)

            def selpk_blk(gi):
                return selpk4[gi // NCHK][:, (gi % NCHK) * 128:
                                          (gi % NCHK) * 128 + 128]

            def wpk_blk(gi):
                return wpk4[gi // NCHK][:, (gi % NCHK) * 128:
                                        (gi % NCHK) * 128 + 128]

            # ---- main loop: 1024-col chunks, 2-tile software pipeline ----
            # PE program order interleaves stage-1 of tile i+1/i+2 between
            # stage-1(i) and stage-2(i) so the PE streams while the clamp
            # (DVE/ACT) catches up; without the lag the PE stalls ~500ns
            # per tile waiting for h.
            LAG = 8
            for cp in range(NS // (2 * CH)):
                oc_map = {}

                def emit_s2(g, t, h):
                    gi = g * TPG + t
                    oc2 = oc_map[g]
                    for half in range(2):
                        nc.tensor.matmul(
                            oc2[half][:],
                            wpk_blk(gi),
                            h[:, half * CH:(half + 1) * CH],
                            start=(t == 0), stop=(t == TPG - 1),
                        )
                    if t == TPG - 1:
                        osb = opool.tile([128, 2 * CH], bf16, tag="osb",
                                         name="osb")
                        nc.scalar.activation(osb[:, 0:CH], oc2[0][:], Ident,
                                             bias=obias[:, g:g + 1])
                        if g % 4 == 1:
                            nc.vector.tensor_scalar(osb[:, CH:2 * CH],
                                                    oc2[1][:],
                                                    obias[:, g:g + 1], None,
                                                    aadd)
                        else:
                            nc.scalar.activation(osb[:, CH:2 * CH],
                                                 oc2[1][:], Ident,
                                                 bias=obias[:, g:g + 1])
                        nc.sync.dma_start(
                            out=out_ext[g * 128:(g + 1) * 128,
                                        2 * cp * CH:2 * (cp + 1) * CH],
                            in_=osb[:])
                        del oc_map[g]

                pend = []
                pend_min = []
                for g in range(NG):
                    xt = xT[g // 2]
                    oc_map[g] = (ocpool.tile([128, CH], f32, tag="oca",
                                             name="oca"),
                                 ocpool.tile([128, CH], f32, tag="ocb",
                                             name="ocb"))
                    for t in range(TPG):
                        gi = g * TPG + t
                        ra = tile_route_a(gi)
                        ps = bcpool.tile([128, 2 * CH], f32, tag="ps")
                        for half in range(2):
                            c = 2 * cp + half
                            nc.tensor.matmul(
                                ps[:, half * CH:(half + 1) * CH],
                                selpk_blk(gi),
                                xt[:, c * CH:(c + 1) * CH],
                                start=True, stop=True,
                            )
                        h = hpool.tile([128, 2 * CH], bf16, tag="h")
                        if ra:
                            nc.vector.tensor_scalar(
                                h[:], ps[:], maxv[:, gi:gi + 1], minv[:, gi:gi + 1],
                                amax, amin,
                            )
                            while len(pend_min) > 1:
                                hq, rq, gq = pend_min.pop(0)
                                nc.vector.tensor_scalar(
                                    hq[:], rq[:], minv[:, gq:gq + 1], None,
                                    amin)
                        else:
                            r = spool.tile([128, 2 * CH], bf16, tag="r")
                            nc.scalar.activation(r[:], ps[:], Relu)
                            while len(pend_min) > 1:
                                hq, rq, gq = pend_min.pop(0)
                                nc.vector.tensor_scalar(
                                    hq[:], rq[:], minv[:, gq:gq + 1], None,
                                    amin)
                            pend_min.append((h, r, gi))
                        pend.append((g, t, h))
                        if len(pend) > LAG:
                            emit_s2(*pend.pop(0))
                for hq, rq, gq in pend_min:
                    nc.vector.tensor_scalar(hq[:], rq[:],
                                            minv[:, gq:gq + 1], None, amin)
                for args in pend:
                    emit_s2(*args)

    nc.compile()
    return nc


def host_constants(edges, width, W, b):
    """Build packed constant tensors. edges/width [F,B], W [F,B,D], b [F,D]."""
    f32 = np.float32
    edges = np.asarray(edges, f32)
    width = np.asarray(width, f32)
    W = np.asarray(W, f32).copy()
    b = np.asarray(b, f32)
    wv_all = bfr(1.0 / width)        # bf16-valued winv, f32
    e1 = edges[:, 0] + width[:, 0]   # second boundary

    selpk = np.zeros((128, NT * 128), f32)
    wpack = np.zeros((128, NT * 128), f32)
    obias = np.zeros((128, NG), f32)
    maxv = np.zeros((128, NT), f32)
    minv = np.zeros((128, NT), f32)

    for g in range(NG):
        ti, band = g // 2, 64 * (g % 2)
        for t in range(TPG):
            gi = g * TPG + t
            ra = tile_route_a(gi)
            selcol = gi * 128
            for m in range(128):
                r = 128 * t + m          # row within the group (0..767)
                fl, j = r // B, r % B    # local feature, bin
                f = 16 * g + fl
                wv = wv_all[f, j]
                wcoef = W[f, j, :]
                if j == 0 and not ra:
                    # relu-form: value = relu((e1-x)*winv), weight -W0,
                    # obias += W0
                    xw = -wv
                    cval = f32(e1[f] * wv)
                    minv[m, gi] = BIG
                    wcoef = -W[f, j, :]
                    obias[8 * fl:8 * fl + 8, g] += W[f, j, :]
                else:
                    xw = wv
                    cval = f32(-edges[f, j] * wv)
                    if j == 0:           # route A bin0: min(s,1) only
                        maxv[m, gi] = -BIG
                        minv[m, gi] = 1.0
                    elif j == B - 1:     # last bin: max(s,0) only
                        maxv[m, gi] = 0.0
                        minv[m, gi] = BIG
                    else:
                        maxv[m, gi] = 0.0
                        minv[m, gi] = 1.0
                chi = bfr(cval)
                clo = f32(cval - chi)
                selpk[band + fl, selcol + m] = xw
                selpk[band + 16 + fl, selcol + m] = xw
                selpk[band + 32, selcol + m] = chi
                selpk[band + 33, selcol + m] = clo
                wpack[m, gi * 128 + 8 * fl:gi * 128 + 8 * fl + 8] = wcoef
        for fl in range(16):
            obias[8 * fl:8 * fl + 8, g] += b[16 * g + fl, :]

    return {
        "selpk": selpk.astype(BF16),
        "wpack": wpack.astype(BF16),
        "obias": obias,
        "maxv": maxv,
        "minv": minv,
    }


def make_xT(x_core):
    """x_core [NS, F] f32 -> packed [8*128, NS] bf16 (hi/lo split bands)."""
    xT = np.zeros((8 * 128, NS), BF16)
    xt_full = np.ascontiguousarray(x_core.T).astype(np.float32)   # [F, NS]
    xhi = xt_full.astype(BF16)
    xlo = (xt_full - xhi.astype(np.float32)).astype(BF16)
    one = BF16(1.0)
    for g in range(NG):
        base = 128 * (g // 2) + 64 * (g % 2)
        xT[base:base + 16, :] = xhi[16 * g:16 * g + 16, :]
        xT[base + 16:base + 32, :] = xlo[16 * g:16 * g + 16, :]
        xT[base + 32, :] = one
        xT[base + 33, :] = one
    return xT


def make_in_maps(x, edges, width, W, b):
    consts = host_constants(edges, width, W, b)
    x = np.ascontiguousarray(np.asarray(x, dtype=np.float32))
    in_maps = []
    for core in range(NCORES):
        m = dict(consts)
        m["xT"] = make_xT(x[core * NS:(core + 1) * NS, :])
        in_maps.append(m)
    return in_maps


def kernel(x, edges, width, W, b):
    if "nc" not in _cache:
        _cache["nc"] = build_nc()
    nc = _cache["nc"]
    in_maps = make_in_maps(x, edges, width, W, b)
    res = run_bass_kernel_spmd(nc, in_maps, core_ids=list(range(NCORES)))
    outs = []
    for r in res.results:
        o = np.asarray(r["out"])                      # [F*D, NS] bf16
        outs.append(o.astype(np.float32).T)           # [NS, F*D]
    return np.ascontiguousarray(np.concatenate(outs, axis=0))


# revision 20
# speedup vs baseline: 1.1876x; 1.1876x over previous
"""Trainium2 Bass kernel for CompactPiecewiseLinearEmbeddings.

out[n, f*8+d] = sum_b h[n,f,b] * W[f,b,d] + b[f,d]
h = piecewise-linear encoding of x[n,f] over per-feature bins
    (first bin clamp_max(1), middle clamp(0,1), last bin clamp_min(0)).

Strategy (per core; data-parallel over N across 8 cores):
 - All-bf16 PE path (f32r moving data caps the PE clock at 1.2 GHz;
   pure-bf16 streams sustain 2.4 GHz).  x is split hi/lo into two bf16
   rows per feature (bf16*bf16 products are exact in the fp32 PSUM
   accumulate), and the per-bin bias -e*winv is split across two bf16
   ones-rows, so stage-1 matches f32r accuracy.
 - Host packs x into 8 xT tiles [128, NS]: two 34-row bands per tile
   (rows 0/64 +: 16 x_hi, 16 x_lo, ones, ones) serving groups (2i,2i+1).
 - Stage-1 bf16 matmul per (group, tile): s[f,j] = winv*x - e*winv.
   Contraction is always the full 128 partitions with zeros in the
   unused weight rows: mixing partial-band (tile_position) matmuls with
   full-128 ones drops the PE cadence from 216ns to ~322ns per matmul
   (measured), while uniform [128,128,512] shapes sustain the 2.4 GHz
   boost.
 - Clamp s -> h (bf16) via two static routes:
     A (~37%): DVE dual tensor_scalar (max, min) from PSUM.
     B: ACT Relu from PSUM (bin0 rows use the negated relu((e1-x)*winv)
        form with +W0 folded into the output bias), then DVE min-pass.
 - Stage-2 bf16 matmul contracts h against block-diagonal W into
   [128 fd, 1024] PSUM (6 accumulating matmuls per group).
 - Output bias+evac split ACT/DVE (3:1) -> bf16 SBUF -> DMA out in
   [fd, n] layout.  Host transposes/casts to the final [n, fd] f32.
"""
import numpy as np
import ml_dtypes

from concourse import bacc, mybir
from concourse.tile import TileContext
from concourse.bass_utils import run_bass_kernel_spmd

N, F, B, D = 16384, 256, 48, 8
NCORES = 8
NS = N // NCORES          # 2048 rows per core
CH = 512                  # matmul free dim (PSUM bank = 512 f32)
NG = 16                   # feature groups (16 features each)
TPG = 6                   # h-tiles per group (16*48/128)
NT = NG * TPG             # 96 h-tiles
NB = 34                   # band rows: 16 x_hi + 16 x_lo + 2 ones
BIG = 1e30

BF16 = ml_dtypes.bfloat16


def tile_route_a(gi):
    """Static clamp route per h-tile: A = DVE dual clamp, B = ACT relu
    + DVE min.  40% A, evenly spread so DVE-heavy tiles never cluster
    (clusters stall the PE on the ps-ring)."""
    return (gi % 10) in (0, 3, 5, 8)


def bfr(a):
    """Round f32 array to bf16 grid, keep f32."""
    return np.asarray(a, np.float32).astype(BF16).astype(np.float32)


_cache = {}


def build_nc():
    nc = bacc.Bacc("TRN2")
    f32, bf16 = mybir.dt.float32, mybir.dt.bfloat16

    xT_ext = nc.declare_dram_parameter("xT", [8 * 128, NS], bf16, isOutput=False)
    selpk_ext = nc.declare_dram_parameter("selpk", [128, NT * 128], bf16,
                                          isOutput=False)
    wpk_ext = nc.declare_dram_parameter("wpack", [128, NT * 128], bf16,
                                        isOutput=False)
    obias_ext = nc.declare_dram_parameter("obias", [128, NG], f32, isOutput=False)
    maxv_ext = nc.declare_dram_parameter("maxv", [128, NT], f32, isOutput=False)
    minv_ext = nc.declare_dram_parameter("minv", [128, NT], f32, isOutput=False)
    out_ext = nc.declare_dram_parameter("out", [F * D, NS], bf16, isOutput=True)

    Ident = mybir.ActivationFunctionType.Identity
    Relu = mybir.ActivationFunctionType.Relu
    amax, amin = mybir.AluOpType.max, mybir.AluOpType.min
    aadd = mybir.AluOpType.add

    with TileContext(nc) as tc:
        with (
            tc.tile_pool(name="const", bufs=1) as cpool,
            tc.tile_pool(name="hbuf", bufs=14) as hpool,
            tc.tile_pool(name="sbuf2", bufs=6) as spool,
            tc.tile_pool(name="osb", bufs=4) as opool,
            tc.tile_pool(name="bc", bufs=3, space="PSUM") as bcpool,
            tc.tile_pool(name="oc", bufs=1, space="PSUM") as ocpool,
        ):
            # ---- constants ----
            xT = [cpool.tile([128, NS], bf16, tag=f"xT{i}", name=f"xT{i}")
                  for i in range(8)]
            # chunked const tiles, DMA-ordered by first use (deps are
            # tile-granular: one big tile would stall the first matmul on
            # the whole transfer)
            NCHK = NT // 4                      # 24 h-tile blocks per chunk
            selpk4 = [cpool.tile([128, NCHK * 128], bf16, tag=f"selpk{i}",
                                 name=f"selpk{i}") for i in range(4)]
            wpk4 = [cpool.tile([128, NCHK * 128], bf16, tag=f"wpk{i}",
                               name=f"wpk{i}") for i in range(4)]
            obias = cpool.tile([128, NG], f32)
            maxv = cpool.tile([128, NT], f32)
            minv = cpool.tile([128, NT], f32)
            CW = NCHK * 128

            def dma_blk(dst, ext, chunk, lo, n):
                """DMA blocks [lo, lo+n) of a chunk tile."""
                nc.sync.dma_start(
                    out=dst[:, lo * 128:(lo + n) * 128],
                    in_=ext[:, (chunk * NCHK + lo) * 128:
                            (chunk * NCHK + lo + n) * 128])

            def dma_xt(i, half, nsl=1):
                """DMA one column half of an xT tile (sliced across nsl
                queues); half 1 feeds only the second cp iteration and
                can arrive late."""
                HN = NS // 2
                w = HN // nsl
                for s in range(nsl):
                    lo = half * HN + s * w
                    nc.sync.dma_start(
                        out=xT[i][:, lo:lo + w],
                        in_=xT_ext[i * 128:(i + 1) * 128, lo:lo + w])

            # critical-first order: the first group needs maxv/minv,
            # selpk blocks 0-5, xT0 cols 0:1024, wpk blocks 0-5; per-queue
            # bandwidth is ~40 GB/s so critical transfers are sliced across
            # several queues
            dma_blk(selpk4[0], selpk_ext, 0, 0, 3)
            dma_xt(0, 0, 4)
            dma_blk(selpk4[0], selpk_ext, 0, 3, 3)
            nc.sync.dma_start(out=maxv[:], in_=maxv_ext[:])
            nc.sync.dma_start(out=minv[:], in_=minv_ext[:])
            dma_blk(wpk4[0], wpk_ext, 0, 0, 3)
            dma_blk(wpk4[0], wpk_ext, 0, 3, 3)
            nc.sync.dma_start(out=obias[:], in_=obias_ext[:])
            dma_xt(1, 0, 2)
            dma_blk(selpk4[0], selpk_ext, 0, 6, 9)
            dma_blk(selpk4[0], selpk_ext, 0, 15, 9)
            dma_blk(wpk4[0], wpk_ext, 0, 6, 9)
            dma_blk(wpk4[0], wpk_ext, 0, 15, 9)
            for i in (1, 2, 3):
                dma_blk(selpk4[i], selpk_ext, i, 0, NCHK)
                dma_xt(2 * i, 0)
                dma_blk(wpk4[i], wpk_ext, i, 0, NCHK)
                dma_xt(2 * i + 1, 0)
            for i in range(8):
                dma_xt(i, 1)

            def selpk_blk(gi):
                return selpk4[gi // NCHK][:, (gi % NCHK) * 128:
                                          (gi % NCHK) * 128 + 128]

            def wpk_blk(gi):
                return wpk4[gi // NCHK][:, (gi % NCHK) * 128:
                                        (gi % NCHK) * 128 + 128]

            # ---- main loop: 1024-col chunks, 2-tile software pipeline ----
            # PE program order interleaves stage-1 of tile i+1/i+2 between
            # stage-1(i) and stage-2(i) so the PE streams while the clamp
            # (DVE/ACT) catches up; without the lag the PE stalls ~500ns
            # per tile waiting for h.
            LAG = 8
            for cp in range(NS // (2 * CH)):
                oc_map = {}

                def emit_s2(g, t, h):
                    gi = g * TPG + t
                    oc2 = oc_map[g]
                    for half in range(2):
                        nc.tensor.matmul(
                            oc2[half][:],
                            wpk_blk(gi),
                            h[:, half * CH:(half + 1) * CH],
                            start=(t == 0), stop=(t == TPG - 1),
                        )
                    if t == TPG - 1:
                        osb = opool.tile([128, 2 * CH], bf16, tag="osb",
                                         name="osb")
                        nc.scalar.activation(osb[:, 0:CH], oc2[0][:], Ident,
                                             bias=obias[:, g:g + 1])
                        if g % 4 == 1:
                            nc.vector.tensor_scalar(osb[:, CH:2 * CH],
                                                    oc2[1][:],
                                                    obias[:, g:g + 1], None,
                                                    aadd)
                        else:
                            nc.scalar.activation(osb[:, CH:2 * CH],
                                                 oc2[1][:], Ident,
                                                 bias=obias[:, g:g + 1])
                        nc.sync.dma_start(
                            out=out_ext[g * 128:(g + 1) * 128,
                                        2 * cp * CH:2 * (cp + 1) * CH],
                            in_=osb[:])
                        del oc_map[g]

                pend = []
                pend_min = []
                for g in range(NG):
                    xt = xT[g // 2]
                    oc_map[g] = (ocpool.tile([128, CH], f32, tag="oca",
                                             name="oca"),
                                 ocpool.tile([128, CH], f32, tag="ocb",
                                             name="ocb"))
                    for t in range(TPG):
                        gi = g * TPG + t
                        ra = tile_route_a(gi)
                        ps = bcpool.tile([128, 2 * CH], f32, tag="ps")
                        for half in range(2):
                            c = 2 * cp + half
                            nc.tensor.matmul(
                                ps[:, half * CH:(half + 1) * CH],
                                selpk_blk(gi),
                                xt[:, c * CH:(c + 1) * CH],
                                start=True, stop=True,
                            )
                        h = hpool.tile([128, 2 * CH], bf16, tag="h")
                        if ra:
                            nc.vector.tensor_scalar(
                                h[:], ps[:], maxv[:, gi:gi + 1], minv[:, gi:gi + 1],
                                amax, amin,
                            )
                            while pend_min:
                                hq, rq, gq = pend_min.pop(0)
                                nc.vector.tensor_scalar(
                                    hq[:], rq[:], minv[:, gq:gq + 1], None,
                                    amin)
                        else:
                            r = spool.tile([128, 2 * CH], bf16, tag="r")
                            nc.scalar.activation(r[:], ps[:], Relu)
                            while pend_min:
                                hq, rq, gq = pend_min.pop(0)
                                nc.vector.tensor_scalar(
                                    hq[:], rq[:], minv[:, gq:gq + 1], None,
                                    amin)
                            pend_min.append((h, r, gi))
                        pend.append((g, t, h))
                        if len(pend) > LAG:
                            emit_s2(*pend.pop(0))
                for hq, rq, gq in pend_min:
                    nc.vector.tensor_scalar(hq[:], rq[:],
                                            minv[:, gq:gq + 1], None, amin)
                for args in pend:
                    emit_s2(*args)

    nc.compile()
    return nc


def host_constants(edges, width, W, b):
    """Build packed constant tensors. edges/width [F,B], W [F,B,D], b [F,D]."""
    f32 = np.float32
    edges = np.asarray(edges, f32)
    width = np.asarray(width, f32)
    W = np.asarray(W, f32).copy()
    b = np.asarray(b, f32)
    wv_all = bfr(1.0 / width)        # bf16-valued winv, f32
    e1 = edges[:, 0] + width[:, 0]   # second boundary

    selpk = np.zeros((128, NT * 128), f32)
    wpack = np.zeros((128, NT * 128), f32)
    obias = np.zeros((128, NG), f32)
    maxv = np.zeros((128, NT), f32)
    minv = np.zeros((128, NT), f32)

    for g in range(NG):
        ti, band = g // 2, 64 * (g % 2)
        for t in range(TPG):
            gi = g * TPG + t
            ra = tile_route_a(gi)
            selcol = gi * 128
            for m in range(128):
                r = 128 * t + m          # row within the group (0..767)
                fl, j = r // B, r % B    # local feature, bin
                f = 16 * g + fl
                wv = wv_all[f, j]
                wcoef = W[f, j, :]
                if j == 0 and not ra:
                    # relu-form: value = relu((e1-x)*winv), weight -W0,
                    # obias += W0
                    xw = -wv
                    cval = f32(e1[f] * wv)
                    minv[m, gi] = BIG
                    wcoef = -W[f, j, :]
                    obias[8 * fl:8 * fl + 8, g] += W[f, j, :]
                else:
                    xw = wv
                    cval = f32(-edges[f, j] * wv)
                    if j == 0:           # route A bin0: min(s,1) only
                        maxv[m, gi] = -BIG
                        minv[m, gi] = 1.0
                    elif j == B - 1:     # last bin: max(s,0) only
                        maxv[m, gi] = 0.0
                        minv[m, gi] = BIG
                    else:
                        maxv[m, gi] = 0.0
                        minv[m, gi] = 1.0
                chi = bfr(cval)
                clo = f32(cval - chi)
                selpk[band + fl, selcol + m] = xw
                selpk[band + 16 + fl, selcol + m] = xw
                selpk[band + 32, selcol + m] = chi
                selpk[band + 33, selcol + m] = clo
                wpack[m, gi * 128 + 8 * fl:gi * 128 + 8 * fl + 8] = wcoef
        for fl in range(16):
            obias[8 * fl:8 * fl + 8, g] += b[16 * g + fl, :]

    return {
        "selpk": selpk.astype(BF16),
        "wpack": wpack.astype(BF16),
        "obias": obias,
        "maxv": maxv,
        "minv": minv,
    }


def make_xT(x_core):
    """x_core [NS, F] f32 -> packed [8*128, NS] bf16 (hi/lo split bands)."""
    xT = np.zeros((8 * 128, NS), BF16)
    xt_full = np.ascontiguousarray(x_core.T).astype(np.float32)   # [F, NS]
    xhi = xt_full.astype(BF16)
    xlo = (xt_full - xhi.astype(np.float32)).astype(BF16)
    one = BF16(1.0)
    for g in range(NG):
        base = 128 * (g // 2) + 64 * (g % 2)
        xT[base:base + 16, :] = xhi[16 * g:16 * g + 16, :]
        xT[base + 16:base + 32, :] = xlo[16 * g:16 * g + 16, :]
        xT[base + 32, :] = one
        xT[base + 33, :] = one
    return xT


def make_in_maps(x, edges, width, W, b):
    consts = host_constants(edges, width, W, b)
    x = np.ascontiguousarray(np.asarray(x, dtype=np.float32))
    in_maps = []
    for core in range(NCORES):
        m = dict(consts)
        m["xT"] = make_xT(x[core * NS:(core + 1) * NS, :])
        in_maps.append(m)
    return in_maps


def kernel(x, edges, width, W, b):
    if "nc" not in _cache:
        _cache["nc"] = build_nc()
    nc = _cache["nc"]
    in_maps = make_in_maps(x, edges, width, W, b)
    res = run_bass_kernel_spmd(nc, in_maps, core_ids=list(range(NCORES)))
    outs = []
    for r in res.results:
        o = np.asarray(r["out"])                      # [F*D, NS] bf16
        outs.append(o.astype(np.float32).T)           # [NS, F*D]
    return np.ascontiguousarray(np.concatenate(outs, axis=0))


# revision 22
# speedup vs baseline: 1.2211x; 1.0282x over previous
"""Trainium2 Bass kernel for CompactPiecewiseLinearEmbeddings.

out[n, f*8+d] = sum_b h[n,f,b] * W[f,b,d] + b[f,d]
h = piecewise-linear encoding of x[n,f] over per-feature bins
    (first bin clamp_max(1), middle clamp(0,1), last bin clamp_min(0)).

Strategy (per core; data-parallel over N across 8 cores):
 - All-bf16 PE path (f32r moving data caps the PE clock at 1.2 GHz;
   pure-bf16 streams sustain 2.4 GHz).  x is split hi/lo into two bf16
   rows per feature (bf16*bf16 products are exact in the fp32 PSUM
   accumulate), and the per-bin bias -e*winv is split across two bf16
   ones-rows, so stage-1 matches f32r accuracy.
 - Host packs x into 8 xT tiles [128, NS]: two 34-row bands per tile
   (rows 0/64 +: 16 x_hi, 16 x_lo, ones, ones) serving groups (2i,2i+1).
 - Stage-1 bf16 matmul per (group, tile): s[f,j] = winv*x - e*winv.
   Contraction is always the full 128 partitions with zeros in the
   unused weight rows: mixing partial-band (tile_position) matmuls with
   full-128 ones drops the PE cadence from 216ns to ~322ns per matmul
   (measured), while uniform [128,128,512] shapes sustain the 2.4 GHz
   boost.
 - Clamp s -> h (bf16) via two static routes:
     A (~37%): DVE dual tensor_scalar (max, min) from PSUM.
     B: ACT Relu from PSUM (bin0 rows use the negated relu((e1-x)*winv)
        form with +W0 folded into the output bias), then DVE min-pass.
 - Stage-2 bf16 matmul contracts h against block-diagonal W into
   [128 fd, 1024] PSUM (6 accumulating matmuls per group).
 - Output bias+evac split ACT/DVE (3:1) -> bf16 SBUF -> DMA out in
   [fd, n] layout.  Host transposes/casts to the final [n, fd] f32.
"""
import numpy as np
import ml_dtypes

from concourse import bacc, mybir
from concourse.tile import TileContext
from concourse.bass_utils import run_bass_kernel_spmd

N, F, B, D = 16384, 256, 48, 8
NCORES = 8
NS = N // NCORES          # 2048 rows per core
CH = 512                  # matmul free dim (PSUM bank = 512 f32)
NG = 16                   # feature groups (16 features each)
TPG = 6                   # h-tiles per group (16*48/128)
NT = NG * TPG             # 96 h-tiles
NB = 34                   # band rows: 16 x_hi + 16 x_lo + 2 ones
BIG = 1e30

BF16 = ml_dtypes.bfloat16
E4M3 = ml_dtypes.float8_e4m3


def tile_route_a(gi):
    """Static clamp route per h-tile: A = DVE dual clamp, B = ACT relu
    + DVE min.  40% A, evenly spread so DVE-heavy tiles never cluster
    (clusters stall the PE on the ps-ring)."""
    return (gi % 10) in (0, 3, 5, 8)


def bfr(a):
    """Round f32 array to bf16 grid, keep f32."""
    return np.asarray(a, np.float32).astype(BF16).astype(np.float32)


_cache = {}


def build_nc():
    nc = bacc.Bacc("TRN2")
    f32, bf16 = mybir.dt.float32, mybir.dt.bfloat16

    xT_ext = nc.declare_dram_parameter("xT", [8 * 128, NS], bf16, isOutput=False)
    selpk_ext = nc.declare_dram_parameter("selpk", [128, NT * 128], bf16,
                                          isOutput=False)
    fp8 = mybir.dt.float8e4
    DRm = mybir.MatmulPerfMode.DoubleRow
    wpk_ext = nc.declare_dram_parameter("wpack", [128, NG * 2 * 128], bf16,
                                        isOutput=False)
    wdr_ext = nc.declare_dram_parameter("wdr", [128, NG * 4, 128], fp8,
                                        isOutput=False)
    obias_ext = nc.declare_dram_parameter("obias", [128, NG], f32, isOutput=False)
    maxv_ext = nc.declare_dram_parameter("maxv", [128, NT], f32, isOutput=False)
    minv_ext = nc.declare_dram_parameter("minv", [128, NT], f32, isOutput=False)
    out_ext = nc.declare_dram_parameter("out", [F * D, NS], bf16, isOutput=True)

    Ident = mybir.ActivationFunctionType.Identity
    Relu = mybir.ActivationFunctionType.Relu
    amax, amin = mybir.AluOpType.max, mybir.AluOpType.min
    aadd = mybir.AluOpType.add

    with TileContext(nc) as tc:
        with (
            tc.tile_pool(name="const", bufs=1) as cpool,
            tc.tile_pool(name="hbuf", bufs=14) as hpool,
            tc.tile_pool(name="sbuf2", bufs=6) as spool,
            tc.tile_pool(name="osb", bufs=4) as opool,
            tc.tile_pool(name="bc", bufs=3, space="PSUM") as bcpool,
            tc.tile_pool(name="oc", bufs=1, space="PSUM") as ocpool,
        ):
            # ---- constants ----
            xT = [cpool.tile([128, NS], bf16, tag=f"xT{i}", name=f"xT{i}")
                  for i in range(8)]
            # chunked const tiles, DMA-ordered by first use (deps are
            # tile-granular: one big tile would stall the first matmul on
            # the whole transfer)
            NCHK = NT // 4                      # 24 h-tile blocks per chunk
            selpk4 = [cpool.tile([128, NCHK * 128], bf16, tag=f"selpk{i}",
                                 name=f"selpk{i}") for i in range(4)]
            wpk4 = [cpool.tile([128, NG * 128], bf16, tag=f"wpk{i}",
                               name=f"wpk{i}") for i in range(2)]
            wdr4 = [cpool.tile([128, NG * 2, 128], fp8, tag=f"wdr{i}",
                               name=f"wdr{i}") for i in range(2)]
            obias = cpool.tile([128, NG], f32)
            maxv = cpool.tile([128, NT], f32)
            minv = cpool.tile([128, NT], f32)
            CW = NCHK * 128

            def dma_blk(dst, ext, chunk, lo, n):
                """DMA blocks [lo, lo+n) of a chunk tile."""
                nc.sync.dma_start(
                    out=dst[:, lo * 128:(lo + n) * 128],
                    in_=ext[:, (chunk * NCHK + lo) * 128:
                            (chunk * NCHK + lo + n) * 128])

            def dma_xt(i, half, nsl=1):
                """DMA one column half of an xT tile (sliced across nsl
                queues); half 1 feeds only the second cp iteration and
                can arrive late."""
                HN = NS // 2
                w = HN // nsl
                for s in range(nsl):
                    lo = half * HN + s * w
                    nc.sync.dma_start(
                        out=xT[i][:, lo:lo + w],
                        in_=xT_ext[i * 128:(i + 1) * 128, lo:lo + w])

            # critical-first order: the first group needs maxv/minv,
            # selpk blocks 0-5, xT0 cols 0:1024, wpk blocks 0-5; per-queue
            # bandwidth is ~40 GB/s so critical transfers are sliced across
            # several queues
            dma_blk(selpk4[0], selpk_ext, 0, 0, 3)
            dma_xt(0, 0, 4)
            dma_blk(selpk4[0], selpk_ext, 0, 3, 3)
            nc.sync.dma_start(out=maxv[:], in_=maxv_ext[:])
            nc.sync.dma_start(out=minv[:], in_=minv_ext[:])
            nc.sync.dma_start(out=wdr4[0][:, 0:4, :], in_=wdr_ext[:, 0:4, :])
            nc.sync.dma_start(out=wpk4[0][:, 0:2 * 128],
                              in_=wpk_ext[:, 0:2 * 128])
            nc.sync.dma_start(out=obias[:], in_=obias_ext[:])
            dma_xt(1, 0, 2)
            dma_blk(selpk4[0], selpk_ext, 0, 6, 9)
            dma_blk(selpk4[0], selpk_ext, 0, 15, 9)
            nc.sync.dma_start(out=wdr4[0][:, 4:NG * 2, :],
                              in_=wdr_ext[:, 4:NG * 2, :])
            nc.sync.dma_start(out=wpk4[0][:, 2 * 128:NG * 128],
                              in_=wpk_ext[:, 2 * 128:NG * 128])
            nc.sync.dma_start(out=wdr4[1][:], in_=wdr_ext[:, NG * 2:, :])
            nc.sync.dma_start(out=wpk4[1][:], in_=wpk_ext[:, NG * 128:])
            for i in (1, 2, 3):
                dma_blk(selpk4[i], selpk_ext, i, 0, NCHK)
                dma_xt(2 * i, 0)
                dma_xt(2 * i + 1, 0)
            for i in range(8):
                dma_xt(i, 1)

            def selpk_blk(gi):
                return selpk4[gi // NCHK][:, (gi % NCHK) * 128:
                                          (gi % NCHK) * 128 + 128]

            def wpk_blk(g, t4):
                b = (g % 8) * 2 + t4
                return wpk4[g // 8][:, b * 128:(b + 1) * 128]

            def wdr_blk(g, u):
                i = (g % 8) * 4 + u * 2
                return wdr4[g // 8][:, i:i + 2, :]

            # ---- main loop: 1024-col chunks, 2-tile software pipeline ----
            # PE program order interleaves stage-1 of tile i+1/i+2 between
            # stage-1(i) and stage-2(i) so the PE streams while the clamp
            # (DVE/ACT) catches up; without the lag the PE stalls ~500ns
            # per tile waiting for h.
            LAG = 6
            for cp in range(NS // (2 * CH)):
                oc_map = {}

                def emit_s2(g, t, h):
                    oc2 = oc_map[g]
                    for half in range(2):
                        if t in (1, 3):
                            nc.tensor.matmul(
                                oc2[half][:],
                                wdr_blk(g, t // 2),
                                h[:, :, half * CH:(half + 1) * CH],
                                start=(t == 1), stop=False,
                                perf_mode=DRm,
                            )
                        else:
                            nc.tensor.matmul(
                                oc2[half][:],
                                wpk_blk(g, t - 4),
                                h[:, half * CH:(half + 1) * CH],
                                start=False, stop=(t == TPG - 1),
                            )
                    if t == TPG - 1:
                        osb = opool.tile([128, 2 * CH], bf16, tag="osb",
                                         name="osb")
                        nc.scalar.activation(osb[:, 0:CH], oc2[0][:], Ident,
                                             bias=obias[:, g:g + 1])
                        if g % 4 == 1:
                            nc.vector.tensor_scalar(osb[:, CH:2 * CH],
                                                    oc2[1][:],
                                                    obias[:, g:g + 1], None,
                                                    aadd)
                        else:
                            nc.scalar.activation(osb[:, CH:2 * CH],
                                                 oc2[1][:], Ident,
                                                 bias=obias[:, g:g + 1])
                        nc.sync.dma_start(
                            out=out_ext[g * 128:(g + 1) * 128,
                                        2 * cp * CH:2 * (cp + 1) * CH],
                            in_=osb[:])
                        del oc_map[g]

                pend = []
                pend_min = []
                for g in range(NG):
                    xt = xT[g // 2]
                    oc_map[g] = (ocpool.tile([128, CH], f32, tag="oca",
                                             name="oca"),
                                 ocpool.tile([128, CH], f32, tag="ocb",
                                             name="ocb"))
                    for t in range(TPG):
                        gi = g * TPG + t
                        ra = tile_route_a(gi)
                        ps = bcpool.tile([128, 2 * CH], f32, tag="ps")
                        for half in range(2):
                            c = 2 * cp + half
                            nc.tensor.matmul(
                                ps[:, half * CH:(half + 1) * CH],
                                selpk_blk(gi),
                                xt[:, c * CH:(c + 1) * CH],
                                start=True, stop=True,
                            )
                        if t < 4:
                            if t % 2 == 0:
                                h2 = hpool.tile([128, 2, 2 * CH], fp8,
                                                tag="h2", name="h2")
                            hv = h2[:, t % 2, :]
                        else:
                            h = hpool.tile([128, 2 * CH], bf16, tag="h",
                                           name="h")
                            hv = h[:]
                        if ra:
                            nc.vector.tensor_scalar(
                                hv, ps[:], maxv[:, gi:gi + 1], minv[:, gi:gi + 1],
                                amax, amin,
                            )
                            while pend_min:
                                hq, rq, gq = pend_min.pop(0)
                                nc.vector.tensor_scalar(
                                    hq, rq[:], minv[:, gq:gq + 1], None,
                                    amin)
                        else:
                            r = spool.tile([128, 2 * CH], bf16, tag="r")
                            nc.scalar.activation(r[:], ps[:], Relu)
                            while pend_min:
                                hq, rq, gq = pend_min.pop(0)
                                nc.vector.tensor_scalar(
                                    hq, rq[:], minv[:, gq:gq + 1], None,
                                    amin)
                            pend_min.append((hv, r, gi))
                        if t in (1, 3):
                            pend.append((g, t, h2))
                        elif t >= 4:
                            pend.append((g, t, h))
                        if len(pend) > LAG:
                            emit_s2(*pend.pop(0))
                for hq, rq, gq in pend_min:
                    nc.vector.tensor_scalar(hq, rq[:],
                                            minv[:, gq:gq + 1], None, amin)
                for args in pend:
                    emit_s2(*args)

    nc.compile()
    return nc


def host_constants(edges, width, W, b):
    """Build packed constant tensors. edges/width [F,B], W [F,B,D], b [F,D]."""
    f32 = np.float32
    edges = np.asarray(edges, f32)
    width = np.asarray(width, f32)
    W = np.asarray(W, f32).copy()
    b = np.asarray(b, f32)
    wv_all = bfr(1.0 / width)        # bf16-valued winv, f32
    e1 = edges[:, 0] + width[:, 0]   # second boundary

    E4 = ml_dtypes.float8_e4m3
    selpk = np.zeros((128, NT * 128), f32)
    wpack = np.zeros((128, NG * 2 * 128), f32)     # bf16 tiles 4,5
    wdr = np.zeros((128, NG * 4, 128), f32)        # fp8 DR pairs, tiles 0-3
    obias = np.zeros((128, NG), f32)
    maxv = np.zeros((128, NT), f32)
    minv = np.zeros((128, NT), f32)

    # Row permutation within each group: middle bins (j=1..46) first so
    # tiles 0-3 contain only h in [0,1] (fp8-safe); unbounded bin0/bin47
    # rows land in tile 5 (bf16).
    def rowmap(fl, j):
        if j == 0:
            return 736 + fl
        if j == B - 1:
            return 752 + fl
        return fl * 46 + (j - 1)

    for g in range(NG):
        ti, band = g // 2, 64 * (g % 2)
        for fl in range(16):
            f = 16 * g + fl
            carry = np.zeros(D, f32)
            for j in range(B):
                row = rowmap(fl, j)
                tt, m = row // 128, row % 128
                gi = g * TPG + tt
                ra = tile_route_a(gi)
                wv = wv_all[f, j]
                wcoef = W[f, j, :]
                if j == 0 and not ra:
                    # relu-form: value = relu((e1-x)*winv), weight -W0,
                    # obias += W0
                    xw = -wv
                    cval = f32(e1[f] * wv)
                    minv[m, gi] = BIG
                    wcoef = -W[f, j, :]
                    obias[8 * fl:8 * fl + 8, g] += W[f, j, :]
                else:
                    xw = wv
                    cval = f32(-edges[f, j] * wv)
                    if j == 0:           # route A bin0: min(s,1) only
                        maxv[m, gi] = -BIG
                        minv[m, gi] = 1.0
                    elif j == B - 1:     # last bin: max(s,0) only
                        maxv[m, gi] = 0.0
                        minv[m, gi] = BIG
                    else:
                        maxv[m, gi] = 0.0
                        minv[m, gi] = 1.0
                chi = bfr(cval)
                clo = f32(cval - chi)
                selcol = gi * 128
                selpk[band + fl, selcol + m] = xw
                selpk[band + 16 + fl, selcol + m] = xw
                selpk[band + 32, selcol + m] = chi
                selpk[band + 33, selcol + m] = clo
                if row < 512:
                    # e4m3 with error feedback along j: keeps the
                    # prefix-sum (saturated-bin) error within one quantum
                    tq = W[f, j, :] + carry
                    q = tq.astype(E4).astype(f32)
                    carry = tq - q
                    wdr[m, g * 4 + (tt // 2) * 2 + (tt % 2),
                        8 * fl:8 * fl + 8] = q
                else:
                    wpack[m, (g * 2 + (tt - 4)) * 128 + 8 * fl:
                          (g * 2 + (tt - 4)) * 128 + 8 * fl + 8] = wcoef
        for fl in range(16):
            obias[8 * fl:8 * fl + 8, g] += b[16 * g + fl, :]

    return {
        "selpk": selpk.astype(BF16),
        "wpack": wpack.astype(BF16),
        "wdr": wdr.astype(E4M3),
        "obias": obias,
        "maxv": maxv,
        "minv": minv,
    }


def make_xT(x_core):
    """x_core [NS, F] f32 -> packed [8*128, NS] bf16 (hi/lo split bands)."""
    xT = np.zeros((8 * 128, NS), BF16)
    xt_full = np.ascontiguousarray(x_core.T).astype(np.float32)   # [F, NS]
    xhi = xt_full.astype(BF16)
    xlo = (xt_full - xhi.astype(np.float32)).astype(BF16)
    one = BF16(1.0)
    for g in range(NG):
        base = 128 * (g // 2) + 64 * (g % 2)
        xT[base:base + 16, :] = xhi[16 * g:16 * g + 16, :]
        xT[base + 16:base + 32, :] = xlo[16 * g:16 * g + 16, :]
        xT[base + 32, :] = one
        xT[base + 33, :] = one
    return xT


def make_in_maps(x, edges, width, W, b):
    consts = host_constants(edges, width, W, b)
    x = np.ascontiguousarray(np.asarray(x, dtype=np.float32))
    in_maps = []
    for core in range(NCORES):
        m = dict(consts)
        m["xT"] = make_xT(x[core * NS:(core + 1) * NS, :])
        in_maps.append(m)
    return in_maps


def kernel(x, edges, width, W, b):
    if "nc" not in _cache:
        _cache["nc"] = build_nc()
    nc = _cache["nc"]
    in_maps = make_in_maps(x, edges, width, W, b)
    res = run_bass_kernel_spmd(nc, in_maps, core_ids=list(range(NCORES)))
    outs = []
    for r in res.results:
        o = np.asarray(r["out"])                      # [F*D, NS] bf16
        outs.append(o.astype(np.float32).T)           # [NS, F*D]
    return np.ascontiguousarray(np.concatenate(outs, axis=0))
